# revision 34
# baseline (speedup 1.0000x reference)
"""Trainium2 Bass kernel for nn_Block_9328668967161.

Computes y = relu(LN_seq(x) @ W1 + b1) @ W2 + b2 + x  where LN_seq
normalizes over the sequence axis (dim 1) with unbiased variance.

Sharding: pure data parallel over the batch axis (32 -> 8 cores x 4).

Per-core pipeline (per batch of [T=2048, C=256]):
  1. DMA x fp32 in a block-token layout (partition p holds tokens
     [16p,16p+16)) so loads/stores are contiguous 16KB lines; cast to bf16
     (split VectorE/ScalarE).
  2. PE-transpose bf16 tiles -> xT [ch, tok] (channel-major), staged in
     PSUM, copied to SBUF by DVE.
  3. LN over seq = free-axis reduction in channel-major: bn_stats/bn_aggr,
     then hT = scale*xT + shift via one DVE tensor_scalar (per-partition
     scale/shift fold gamma/beta/mean/rstd). This chain is priority-boosted
     because it gates mm1.
  4. mm1: h1T[dff, tok] = W1.T @ hT (K=ch on partitions), bf16, fp32 PSUM;
     relu+b1 epilogue on ScalarE (per-partition bias), output aT bf16.
  5. mm2: ff[tok, ch] = aT.T @ W2 (K=dff on partitions) -> token-major PSUM.
  6. residual: y = ff_psum + (x + b2) in fp32 (x+b2 precomputed in-place on
     GPSIMD), DMA out. Only ONE transpose (input side) is needed; mm2's
     lhsT=aT trick makes the output land token-major.

Schedule shaping: a PE warm-up block defeats the HAM cold clock at start;
batch b+1's pre-chain is emitted before batch b's matmuls (software
pipelining) so the PE never starves at batch boundaries.
"""

import os
import sys

sys.path.insert(0, "/opt/trn_rl_repo")

import numpy as np

import concourse.bass as bass
import concourse.tile as tile
from concourse import bacc
from concourse import mybir
from concourse.bass_utils import run_bass_kernel_spmd
from concourse.masks import make_identity

B, T, C, D = 32, 2048, 256, 1024
N_CORES = 8
BL = B // N_CORES  # batches per core
EPS = 1e-5
KC = C // 128  # 2 channel chunks
KD = D // 128  # 8 dff chunks
NT = T // 128  # 16 token chunks

f32 = mybir.dt.float32
bf16 = mybir.dt.bfloat16
Alu = mybir.AluOpType
Act = mybir.ActivationFunctionType



def _body(tc, x, gamma, beta, W1, b1, W2, b2, y):
    nc = tc.nc

    from contextlib import ExitStack

    with ExitStack() as ctx:
        consts = ctx.enter_context(tc.tile_pool(name="consts", bufs=1))
        wstage = ctx.enter_context(tc.tile_pool(name="wstage", bufs=1))
        small = ctx.enter_context(tc.tile_pool(name="small", bufs=4))
        xf_pool = ctx.enter_context(tc.tile_pool(name="xf", bufs=3))
        xb_pool = ctx.enter_context(tc.tile_pool(name="xb", bufs=2))
        xT_pool = ctx.enter_context(tc.tile_pool(name="xT", bufs=2))
        hT_pool = ctx.enter_context(tc.tile_pool(name="hT", bufs=2))
        aT_pool = ctx.enter_context(tc.tile_pool(name="aT", bufs=2))
        y_pool = ctx.enter_context(tc.tile_pool(name="ysb", bufs=2))
        psumT = ctx.enter_context(tc.tile_pool(name="psumT", bufs=2, space="PSUM"))
        psum1 = ctx.enter_context(tc.tile_pool(name="psum1", bufs=3, space="PSUM"))
        psum2 = ctx.enter_context(tc.tile_pool(name="psum2", bufs=2, space="PSUM"))
        psumW = ctx.enter_context(tc.tile_pool(name="psumW", bufs=1, space="PSUM"))

        # ---- constants -------------------------------------------------
        ident = consts.tile([128, 128], f32)
        make_identity(nc, ident[:])
        identb = consts.tile([128, 128], bf16)
        make_identity(nc, identb[:])

        # PE warm-up: ~10us of dependency-free dummy matmuls so the HAM
        # clock-gate reaches 8/8 (2.4 GHz) before the first real batch,
        # and the PE stays busy while batch 0 loads.
        psw = psumW.tile([128, 128], f32, tag="psw")
        for _ in range(32):
            nc.tensor.matmul(
                psw[:], lhsT=ident[:], rhs=ident[:], start=True, stop=True
            )

        # Block token layout: partition p holds tokens [16p, 16p+16) so the
        # x load / y store are 128 contiguous 16KB lines per batch (minimal
        # DMA descriptor generation). The token permutation (block-major in
        # SBUF, interleaved in xT's free dim) is self-consistent end to end:
        # LN stats are permutation-invariant, and mm2's m-loop selects
        # within-block index m whose residual slice is exactly xf[:, m, :].
        xv = x.rearrange("b (p i) c -> p b i c", i=NT)
        yv = y.rearrange("b (p i) c -> p b i c", i=NT)

        def load(b):
            """Issue batch b's x load (4 contiguous quarter-batch DMAs)."""
            xf = xf_pool.tile([128, NT, C], f32, tag="xf", name="xf")
            for g in range(4):
                nc.sync.dma_start(
                    out=xf[:, 4 * g : 4 * g + 4, :], in_=xv[:, b, 4 * g : 4 * g + 4, :]
                )
            return xf

        # batch 0's load goes out before the (big) weight DMAs
        xf0 = load(0)

        # single batched DMA per parameter tensor (each dma_start costs
        # ~0.65us of serial Sync-engine issue time)
        w1st = wstage.tile([128, KC, D], f32, tag="w1st")
        nc.sync.dma_start(out=w1st[:], in_=W1.rearrange("(kc p) d -> p kc d", p=128))
        w1sb = []
        for kc in range(KC):
            wt = consts.tile([128, D], bf16, tag=f"w1_{kc}")
            nc.scalar.copy(out=wt[:], in_=w1st[:, kc, :])
            w1sb.append(wt)

        w2st = wstage.tile([128, KD, C], f32, tag="w2st")
        nc.sync.dma_start(out=w2st[:], in_=W2.rearrange("(d p) c -> p d c", p=128))
        w2sb = []
        for d in range(KD):
            wt = consts.tile([128, C], bf16, tag=f"w2_{d}")
            nc.scalar.copy(out=wt[:], in_=w2st[:, d, :])
            w2sb.append(wt)

        gam_t = consts.tile([128, KC], f32, tag="gam")
        nc.gpsimd.dma_start(
            out=gam_t[:], in_=gamma.rearrange("(kc p) o -> p (kc o)", p=128)
        )
        bet_t = consts.tile([128, KC], f32, tag="bet")
        nc.gpsimd.dma_start(
            out=bet_t[:], in_=beta.rearrange("(kc p) o -> p (kc o)", p=128)
        )
        gam = [gam_t[:, kc : kc + 1] for kc in range(KC)]
        bet = [bet_t[:, kc : kc + 1] for kc in range(KC)]

        b1t = consts.tile([128, KD], f32, tag="b1t")
        nc.gpsimd.dma_start(out=b1t[:], in_=b1.rearrange("(d p) o -> p (d o)", p=128))
        b1sb = [b1t[:, d : d + 1] for d in range(KD)]

        # b2 replicated across partitions (DMA broadcast), fp32
        b2rep = consts.tile([128, C], f32, tag="b2rep")
        b2_bcast = bass.AP(tensor=b2.tensor, offset=b2.offset, ap=[[0, 128], [1, C]])
        nc.gpsimd.dma_start(out=b2rep[:], in_=b2_bcast)

        eps_t = consts.tile([128, 1], f32, tag="eps")
        nc.vector.memset(eps_t[:], EPS)

        # ---- per-batch pipeline ---------------------------------------
        def pre(b, xf):
            """Cast + transpose + LN stats + affine for batch b.
            Returns (xf, hT) for the mm stage."""
            xb = xb_pool.tile([128, NT, C], bf16, tag="xb", name="xb")
            for g in range(4):
                nc.scalar.copy(
                    out=xb[:, 4 * g : 4 * g + 4, :], in_=xf[:, 4 * g : 4 * g + 4, :]
                )

            # transpose to channel-major xT[kc] = [128ch, T]; bn_stats per
            # half as soon as its copy lands (shortens the stats latency)
            xT = [
                xT_pool.tile([128, T], bf16, tag=f"xT{kc}", name=f"xT{kc}")
                for kc in range(KC)
            ]
            stats_t = [
                small.tile([128, 4, 6], f32, tag=f"stats{kc}", name=f"stats{kc}")
                for kc in range(KC)
            ]
            for kc in range(KC):
                xTr = xT[kc].rearrange("p (s f) -> p s f", f=512)
                for q in range(4):
                    pt = psumT.tile([128, 512], bf16, tag="psumT", name="pt")
                    for j in range(4):
                        i = q * 4 + j
                        nc.tensor.transpose(
                            out=pt[:, j * 128 : (j + 1) * 128],
                            in_=xb[:, i, kc * 128 : (kc + 1) * 128],
                            identity=identb[:],
                        )
                    with tc.high_priority():
                        nc.vector.tensor_copy(
                            out=xT[kc][:, q * 512 : (q + 1) * 512], in_=pt[:]
                        )
                        nc.vector.bn_stats(
                            out=stats_t[kc][:, q, :], in_=xTr[:, q, :]
                        )

            # x + b2 on GPSIMD (fp32), in place: residual carrier
            for g in range(4):
                nc.gpsimd.tensor_add(
                    out=xf[:, 4 * g : 4 * g + 4, :],
                    in0=xf[:, 4 * g : 4 * g + 4, :],
                    in1=bass.AP(
                        tensor=b2rep[:].tensor,
                        offset=b2rep[:].offset,
                        ap=[b2rep[:].ap[0], [0, 4], b2rep[:].ap[1]],
                    ),
                )

            # LN stats + affine -> hT (bf16)
            hT = [
                hT_pool.tile([128, T], bf16, tag=f"hT{kc}", name=f"hT{kc}")
                for kc in range(KC)
            ]
            for kc in range(KC):
                with tc.high_priority():
                    mv = small.tile([128, 2], f32, tag="mv", name="mv")
                    nc.vector.bn_aggr(out=mv[:], in_=stats_t[kc][:])
                    # std = sqrt(var_pop * T/(T-1) + eps)
                    std = small.tile([128, 1], f32, tag="std", name="std")
                    nc.scalar.activation(
                        out=std[:],
                        in_=mv[:, 1:2],
                        func=Act.Sqrt,
                        bias=eps_t[:],
                        scale=float(T) / (T - 1),
                    )
                    rstd = small.tile([128, 1], f32, tag="rstd", name="rstd")
                    nc.vector.reciprocal(out=rstd[:], in_=std[:])
                    scl = small.tile([128, 1], f32, tag="scl", name="scl")
                    nc.vector.tensor_mul(out=scl[:], in0=rstd[:], in1=gam[kc][:])
                    tmp = small.tile([128, 1], f32, tag="tmp", name="tmp")
                    nc.vector.tensor_mul(out=tmp[:], in0=mv[:, 0:1], in1=scl[:])
                    shf = small.tile([128, 1], f32, tag="shf", name="shf")
                    nc.vector.tensor_sub(out=shf[:], in0=bet[kc][:], in1=tmp[:])
                    nc.vector.tensor_scalar(
                        out=hT[kc][:],
                        in0=xT[kc][:],
                        scalar1=scl[:],
                        scalar2=shf[:],
                        op0=Alu.mult,
                        op1=Alu.add,
                    )
            return xf, hT

        def mm(b, xf, hT):
            """mm1 + relu + mm2 + residual + store for batch b."""
            aT = [
                aT_pool.tile([128, T], bf16, tag=f"aT{d}", name=f"aT{d}")
                for d in range(KD)
            ]
            for d in range(KD):
                for jt in range(4):
                    ps = psum1.tile([128, 512], f32, tag="psum1", name="ps")
                    for kc in range(KC):
                        nc.tensor.matmul(
                            ps[:],
                            lhsT=w1sb[kc][:, d * 128 : (d + 1) * 128],
                            rhs=hT[kc][:, jt * 512 : (jt + 1) * 512],
                            start=(kc == 0),
                            stop=(kc == KC - 1),
                        )
                    # relu + b1, all on ScalarE (DVE is the PE-feeding engine)
                    nc.scalar.activation(
                        out=aT[d][:, jt * 512 : (jt + 1) * 512],
                        in_=ps[:],
                        func=Act.Relu,
                        bias=b1sb[d][:],
                        scale=1.0,
                    )

            # mm2 + residual + store (y staged in quarter-batch tiles so the
            # store is 4 big DMAs instead of 8 small ones)
            xf_flat = xf.rearrange("p n c -> p (n c)")
            for q in range(4):
                ysb = y_pool.tile([128, 1024], f32, tag="ysb", name="ysb")
                for qh in range(2):
                    mp = q * 2 + qh  # pair of token chunks
                    ps2 = psum2.tile([128, 512], f32, tag="psum2", name="ps2")
                    for half in range(2):
                        m = mp * 2 + half
                        for d in range(KD):
                            nc.tensor.matmul(
                                ps2[:, half * 256 : (half + 1) * 256],
                                lhsT=aT[d][:, m * 128 : (m + 1) * 128],
                                rhs=w2sb[d][:],
                                start=(d == 0),
                                stop=(d == KD - 1),
                            )
                    nc.vector.tensor_add(
                        out=ysb[:, qh * 512 : (qh + 1) * 512],
                        in0=ps2[:],
                        in1=xf_flat[:, mp * 512 : (mp + 1) * 512],
                    )
                nc.sync.dma_start(
                    out=yv[:, b, 4 * q : 4 * q + 4, :],
                    in_=ysb.rearrange("p (n c) -> p n c", c=C),
                )

        # software-pipelined emission: batch b+1's pre-chain is emitted
        # before batch b's matmuls so every engine stream interleaves and
        # the PE never starves at batch boundaries. Batch 0's load was
        # issued before the weight loads (xf0); later loads are issued two
        # batches ahead so the bf16 casts never wait on DMA.
        lds = {0: xf0, 1: load(1)}
        state = pre(0, lds.pop(0))
        # filler: keep the PE busy (and the HAM clock warm) while batch 0's
        # LN stats chain finishes on VectorE
        psw2 = psumW.tile([128, 128], f32, tag="psw", name="psw2")
        for _ in range(56):
            nc.tensor.matmul(
                psw2[:], lhsT=identb[:], rhs=identb[:], start=True, stop=True
            )
        for b in range(BL):
            if b + 2 < BL:
                lds[b + 2] = load(b + 2)
            nxt = pre(b + 1, lds.pop(b + 1)) if b + 1 < BL else None
            mm(b, *state)
            state = nxt


_CACHED_NC = None


def _build_nc():
    global _CACHED_NC
    if _CACHED_NC is not None:
        return _CACHED_NC
    nc = bacc.Bacc("TRN2", target_bir_lowering=False, debug=False)
    x_d = nc.dram_tensor("x", [BL, T, C], f32, kind="ExternalInput")
    g_d = nc.dram_tensor("gamma", [C, 1], f32, kind="ExternalInput")
    be_d = nc.dram_tensor("beta", [C, 1], f32, kind="ExternalInput")
    w1_d = nc.dram_tensor("W1", [C, D], f32, kind="ExternalInput")
    b1_d = nc.dram_tensor("b1", [D, 1], f32, kind="ExternalInput")
    w2_d = nc.dram_tensor("W2", [D, C], f32, kind="ExternalInput")
    b2_d = nc.dram_tensor("b2", [1, C], f32, kind="ExternalInput")
    y_d = nc.dram_tensor("y", [BL, T, C], f32, kind="ExternalOutput")
    with tile.TileContext(nc) as tc:
        _body(
            tc,
            x_d.ap(),
            g_d.ap(),
            be_d.ap(),
            w1_d.ap(),
            b1_d.ap(),
            w2_d.ap(),
            b2_d.ap(),
            y_d.ap(),
        )
    nc.finalize()
    _CACHED_NC = nc
    return nc


def run(inputs, trace=False, **kw):
    nc = _build_nc()
    x = np.ascontiguousarray(np.asarray(inputs["x"], dtype=np.float32))
    gamma = np.asarray(inputs["gamma"], dtype=np.float32).reshape(C, 1)
    beta = np.asarray(inputs["beta"], dtype=np.float32).reshape(C, 1)
    W1 = np.ascontiguousarray(np.asarray(inputs["W1"], dtype=np.float32))
    b1 = np.asarray(inputs["b1"], dtype=np.float32).reshape(D, 1)
    W2 = np.ascontiguousarray(np.asarray(inputs["W2"], dtype=np.float32))
    b2 = np.asarray(inputs["b2"], dtype=np.float32).reshape(1, C)

    in_maps = []
    for c in range(N_CORES):
        in_maps.append(
            {
                "x": x[c * BL : (c + 1) * BL],
                "gamma": gamma,
                "beta": beta,
                "W1": W1,
                "b1": b1,
                "W2": W2,
                "b2": b2,
            }
        )
    res = run_bass_kernel_spmd(nc, in_maps, list(range(N_CORES)), trace=trace, **kw)
    y = np.concatenate([res.results[c]["y"] for c in range(N_CORES)], axis=0)
    return y, res


def kernel(**inputs):
    y, _ = run(inputs, trace=False)
    return y
